# revision 3
# baseline (speedup 1.0000x reference)
"""Trainium2 Bass kernel v2 for nn_Attention (qkv proj + RoPE + causal
attention + out proj), tensor-parallel over 8 NeuronCores: core c handles
batch b=c//2, head-group g=c%2 (8 heads).

v2 changes vs baseline:
- bf16 activations/weights end-to-end (fp32 psum accumulation, fp32 rope
  tables, fp32 softmax denominators) -> half the DMA bytes, 2x DVE adds,
  no fp32r small-N matmul penalty.
- pair-ordered pipeline: Q/K for head-pair 0 projected first, so the
  ScalarE exp backbone starts ~25us in instead of ~130us; V projection and
  later pairs' projections fill TensorE idle while exp runs.
- ancillary DMAs (rope swap, l row-move, odd-head output move) issued on
  the Pool queue (25ns issue) instead of SP/ACT queues (600-900ns).
"""

from contextlib import ExitStack

import numpy as np

import concourse.bass as bass
import concourse.tile as tile
from concourse import bacc, mybir
from concourse.bass import ds, ts
from concourse.bass_utils import run_bass_kernel_spmd

B, S, D, H, DH = 4, 2048, 1024, 16, 64
HL = 8          # heads per core
INNER = H * DH  # 1024
KC = D // 128   # 8 contraction chunks
NT = S // 128   # 16 token tiles
F32 = mybir.dt.float32
BF = mybir.dt.bfloat16

EXP = mybir.ActivationFunctionType.Exp
SCALE = 1.0 / np.sqrt(DH)


def _emit_attention(nc, pools, qkt, vsb, ot, pair, parts=None, filler=None):
    """Attention for the two heads of `pair` (local heads 2p, 2p+1)."""
    scp, pjp, pavp, nrm = (pools[k] for k in ("sc", "pj", "pav", "nrm"))
    if parts is None:
        parts = [(h, qh) for h in range(2) for qh in range(2)]
    for h, qh in parts:
        if True:
            hloc = 2 * pair + h
            q_ap = qkt[pair][ds(64 * h, 64), :]
            k_ap = qkt[4 + pair][ds(64 * h, 64), :]
            q0, q1 = 1024 * qh, 1024 * (qh + 1)
            pav = [pavp.tile([DH + 1, 512], F32, tag="pav", name=f"pav{_c}")
                   for _c in range(2)]
            jmax = 8 * (qh + 1) - 1
            for j in range(jmax + 1):
                gs = max(q0, 128 * j)       # first valid q col
                cw = q1 - gs
                ps = scp.tile([128, cw], F32, tag="sc")
                for po in range(0, cw, 512):
                    pw = min(512, cw - po)
                    nc.tensor.matmul(
                        ps[:, ds(po, pw)],
                        k_ap[:, ds(128 * j, 128)],
                        q_ap[:, ds(gs + po, pw)],
                        start=True, stop=True)
                pj = pjp.tile([128, cw], BF, tag="P")
                nc.scalar.activation(pj[:], ps[:], EXP, scale=SCALE)
                if gs == 128 * j:
                    # diagonal block: causal-mask first 128 cols
                    nc.gpsimd.affine_select(
                        out=pj[:, 0:128], in_=pj[:, 0:128],
                        compare_op=mybir.AluOpType.is_ge, fill=0.0,
                        base=0, pattern=[[1, 128]],
                        channel_multiplier=-1)
                # AV accumulate: one psum accumulator per 512 q-cols
                for c in range(2):
                    cr = q0 + 512 * c       # abs start col of region
                    cs = max(cr, 128 * j)   # abs start col this j covers
                    w = cr + 512 - cs
                    if w <= 0:
                        continue
                    nc.tensor.matmul(
                        pav[c][:, ds(cs - cr, w)],
                        vsb[:, j, hloc, 0:DH + 1],
                        pj[:, ds(cs - gs, w)],
                        start=(j == 0),
                        stop=(j == min(jmax, (cr + 511) // 128)))
                if filler is not None:
                    next(filler, None)
            # ---- normalize per region: pav[0:64] / pav[64] -> ot ----
            for c in range(2):
                cr = q0 + 512 * c
                qsl = ds(cr, 512)
                sc = nrm.tile([64, 512], BF, tag="sc")
                nc.vector.tensor_copy(sc[:], pav[c][ds(0, DH), :])
                lf = nrm.tile([128, 512], F32, tag="lf")
                nc.vector.tensor_copy(lf[ds(64, 1), :], pav[c][ds(DH, 1), :])
                nc.sync.dma_start(lf[ds(0, 1), :], lf[ds(64, 1), :])
                nc.vector.reciprocal(lf[ds(0, 1), :], lf[ds(0, 1), :])
                rb = nrm.tile([64, 512], F32, tag="rb")
                nc.gpsimd.partition_broadcast(
                    rb[:], lf[ds(0, 1), :], channels=64)
                if h == 0:
                    nc.vector.tensor_mul(ot[pair][ds(0, 64), qsl], sc[:], rb[:])
                else:
                    ott = nrm.tile([64, 512], BF, tag="ott")
                    nc.vector.tensor_mul(ott[:], sc[:], rb[:])
                    nc.sync.dma_start(ot[pair][ds(64, 64), qsl], ott[:])


def build_kernel(nc, phases=3):
    xT = nc.dram_tensor("xT", [D, S], BF, kind="ExternalInput").ap()
    wq = nc.dram_tensor("wq", [D, HL * DH], BF, kind="ExternalInput").ap()
    wk = nc.dram_tensor("wk", [D, HL * DH], BF, kind="ExternalInput").ap()
    wv = nc.dram_tensor("wv", [D, HL * DH], BF, kind="ExternalInput").ap()
    wo = nc.dram_tensor("wo", [HL * DH, D], BF, kind="ExternalInput").ap()
    cc = nc.dram_tensor("cc", [128, S], BF, kind="ExternalInput").ap()
    ssw = nc.dram_tensor("ssw", [128, S], BF, kind="ExternalInput").ap()
    y = nc.dram_tensor("y", [S, D], BF, kind="ExternalOutput").ap()

    with tile.TileContext(nc) as tc, ExitStack() as top:
        cpool = top.enter_context(tc.tile_pool(name="consts", bufs=1))
        qkpool = top.enter_context(tc.tile_pool(name="qkp", bufs=1))
        otpool = top.enter_context(tc.tile_pool(name="otp", bufs=1))

        # ---- input loads (SP queue), most-urgent first ----
        wq_sb = cpool.tile([128, KC, 512], BF, tag="wq", name="wq")
        nc.sync.dma_start(wq_sb[:], wq.rearrange("(k p) n -> p k n", p=128))
        cc_sb = cpool.tile([128, S], BF, tag="cc", name="cc")
        nc.sync.dma_start(cc_sb[:], cc[:, :])
        ssw_sb = cpool.tile([128, S], BF, tag="ssw", name="ssw")
        nc.sync.dma_start(ssw_sb[:], ssw[:, :])
        xsb = []
        for k in range(KC):
            xh = cpool.tile([128, S], BF, tag=f"x{k}", name=f"x{k}")
            nc.sync.dma_start(xh[:], xT[ts(k, 128), :])
            xsb.append(xh)
        wk_sb = cpool.tile([128, KC, 512], BF, tag="wk", name="wk")
        nc.sync.dma_start(wk_sb[:], wk.rearrange("(k p) n -> p k n", p=128))
        wv_sb = cpool.tile([128, KC, 512], BF, tag="wv", name="wv")
        nc.sync.dma_start(wv_sb[:], wv.rearrange("(k p) n -> p k n", p=128))
        wo_sb = cpool.tile([128, 4, D], BF, tag="wo", name="wo")
        nc.sync.dma_start(wo_sb[:], wo.rearrange("(k p) n -> p k n", p=128))

        qkt = [qkpool.tile([128, S], BF, tag=f"qkt{t}", name=f"qkt{t}")
               for t in range(8)]
        vsb = qkpool.tile([128, NT, HL, DH + 1], BF, tag="vsb", name="vsb")
        ot = [otpool.tile([128, S], BF, tag=f"ot{t}", name=f"ot{t}")
              for t in range(4)]

        nc.gpsimd.memset(vsb[:, :, :, DH], 1.0)
        # pre-warm the exp table set while projections run
        warm = cpool.tile([1, 16], F32, tag="warm", name="warm")
        nc.gpsimd.memset(warm[:], 0.0)
        nc.scalar.activation(warm[:], warm[:], EXP, scale=1.0)

        with ExitStack() as mid:
            rtmp = mid.enter_context(tc.tile_pool(name="rtmp", bufs=4))
            scp = mid.enter_context(
                tc.tile_pool(name="scp", bufs=2, space="PSUM"))
            pjp = mid.enter_context(tc.tile_pool(name="pjp", bufs=6))
            pavp = mid.enter_context(
                tc.tile_pool(name="pavp", bufs=2, space="PSUM"))
            nrm = mid.enter_context(tc.tile_pool(name="nrm", bufs=2))
            projscope = mid.enter_context(ExitStack())
            psp = projscope.enter_context(
                tc.tile_pool(name="psp", bufs=2, space="PSUM"))
            pools = dict(sc=scp, pj=pjp, pav=pavp, nrm=nrm)

            def project_qk(pair, chs=range(4)):
                """Q,K projection + rope for one head-pair, 512-col chunks."""
                for wsb, toff in ((wq_sb, pair), (wk_sb, 4 + pair)):
                    for ch in chs:            # 512-token chunks
                        off = 512 * ch
                        ps = psp.tile([128, 512], F32, tag="ps512")
                        for k in range(KC):
                            nc.tensor.matmul(
                                ps[:], wsb[:, k, ts(pair, 128)],
                                xsb[k][:, ds(off, 512)],
                                start=(k == 0), stop=(k == KC - 1))
                        sl = ds(off, 512)
                        nc.vector.tensor_mul(
                            qkt[toff][:, sl], ps[:], cc_sb[:, sl])
                        v2 = rtmp.tile([128, 512], BF, tag="v2")
                        nc.vector.tensor_mul(v2[:], ps[:], ssw_sb[:, sl])
                        v2s = rtmp.tile([128, 512], BF, tag="v2s")
                        for blk in range(4):
                            src = (blk ^ 1) * 32
                            nc.sync.dma_start(
                                v2s[ds(blk * 32, 32), :], v2[ds(src, 32), :])
                        nc.vector.tensor_tensor(
                            qkt[toff][:, sl], qkt[toff][:, sl], v2s[:],
                            op=mybir.AluOpType.add)

            def project_v(tts):
                for tt in tts:
                    psV = psp.tile([128, 512], F32, tag="ps512")
                    for k in range(KC):
                        nc.tensor.matmul(
                            psV[:], xsb[k][:, ts(tt, 128)], wv_sb[:, k, :],
                            start=(k == 0), stop=(k == KC - 1))
                    nc.scalar.copy(
                        vsb[:, tt, :, 0:DH],
                        psV[:].rearrange("p (h d) -> p h d", h=HL))

            project_qk(0, chs=[0, 1])
            if phases >= 2:
                project_v(range(0, 8))
                _emit_attention(nc, pools, qkt, vsb, ot, 0, parts=[(0, 0)])
                project_qk(0, chs=[2, 3])
                project_v(range(8, 16))
                _emit_attention(nc, pools, qkt, vsb, ot, 0,
                                parts=[(0, 1), (1, 0), (1, 1)])
                for pair in range(1, 4):
                    project_qk(pair)
                    _emit_attention(nc, pools, qkt, vsb, ot, pair)
            else:
                project_v(range(0, 8))
                project_qk(0, chs=[2, 3])
                project_v(range(8, 16))
                for pair in range(1, 4):
                    project_qk(pair)

        # ---------------- out projection -----------------------------
        if phases < 3:
            return nc
        with ExitStack() as ph:
            ypool = ph.enter_context(tc.tile_pool(name="ypool", bufs=3))
            psy = ph.enter_context(
                tc.tile_pool(name="psy", bufs=3, space="PSUM"))
            for tt in range(NT):
                ps = psy.tile([128, D], F32, tag="psy")
                for k in range(4):
                    for half in range(2):
                        nc.tensor.matmul(
                            ps[:, ts(half, 512)],
                            ot[k][:, ts(tt, 128)],
                            wo_sb[:, k, ts(half, 512)],
                            start=(k == 0), stop=(k == 3))
                ysb = ypool.tile([128, D], BF, tag="y")
                nc.scalar.copy(ysb[:], ps[:])
                nc.sync.dma_start(y[ts(tt, 128), :], ysb[:])
    return nc


# ---------------- host side ------------------------------------------------

def _rope_tables():
    i = np.arange(DH // 2, dtype=np.float32)
    thetas = np.power(np.float32(10000.0), -2.0 * (i - 1.0) / DH)
    vals = thetas[:, None].astype(np.float32) * \
        np.arange(S, dtype=np.float32)[None, :]
    cos32 = np.cos(vals).astype(np.float32)
    sin32 = np.sin(vals).astype(np.float32)
    CC = np.tile(cos32, (4, 1))
    SSsw = np.concatenate([sin32, -sin32, sin32, -sin32], axis=0)
    return np.ascontiguousarray(CC), np.ascontiguousarray(SSsw)


def _qk_col_perm(g):
    cols = []
    for m in range(4):
        for hh in (2 * m, 2 * m + 1):
            hg = HL * g + hh
            cols += [hg * DH + 2 * i for i in range(32)]
            cols += [hg * DH + 2 * i + 1 for i in range(32)]
    return np.array(cols)


_CACHE = {}


def _get_module():
    if "nc" not in _CACHE:
        nc = bacc.Bacc("TRN2", target_bir_lowering=False, debug=False,
                       num_devices=8)
        build_kernel(nc)
        nc.compile()
        _CACHE["nc"] = nc
    return _CACHE["nc"]


def make_in_maps(x, Wqkv, Wout):
    import ml_dtypes
    bf16 = ml_dtypes.bfloat16
    x = np.asarray(x, np.float32)
    Wqkv = np.asarray(Wqkv, np.float32)
    Wout = np.asarray(Wout, np.float32)
    CC, SSsw = _rope_tables()
    shard = {}
    for g in range(2):
        perm = _qk_col_perm(g)
        vcols = np.arange(HL * g * DH, HL * (g + 1) * DH)
        shard[g] = dict(
            wq=np.ascontiguousarray(
                Wqkv[:, 0 * INNER:1 * INNER][:, perm].astype(bf16)),
            wk=np.ascontiguousarray(
                Wqkv[:, 1 * INNER:2 * INNER][:, perm].astype(bf16)),
            wv=np.ascontiguousarray(
                Wqkv[:, 2 * INNER:3 * INNER][:, vcols].astype(bf16)),
            wo=np.ascontiguousarray(Wout[vcols, :].astype(bf16)),
        )
    in_maps = []
    for c in range(8):
        b, g = c // 2, c % 2
        in_maps.append(dict(
            xT=np.ascontiguousarray(x[b].T.astype(bf16)),
            cc=CC.astype(bf16), ssw=SSsw.astype(bf16), **shard[g]))
    return in_maps


def kernel(x, Wqkv, Wout, bout):
    bout = np.asarray(bout, np.float32)
    nc = _get_module()
    in_maps = make_in_maps(x, Wqkv, Wout)
    res = run_bass_kernel_spmd(nc, in_maps, core_ids=list(range(8)))
    ys = [np.asarray(r["y"], dtype=np.float32) for r in res.results]
    out = np.stack([ys[2 * b] + ys[2 * b + 1] + bout for b in range(B)])
    return out.astype(np.float32)


# revision 4
# speedup vs baseline: 1.4499x; 1.4499x over previous
"""Trainium2 Bass kernel v2 for nn_Attention (qkv proj + RoPE + causal
attention + out proj), tensor-parallel over 8 NeuronCores: core c handles
batch b=c//2, head-group g=c%2 (8 heads).

v2 changes vs baseline:
- bf16 activations/weights end-to-end (fp32 psum accumulation, fp32 rope
  tables, fp32 softmax denominators) -> half the DMA bytes, 2x DVE adds,
  no fp32r small-N matmul penalty.
- pair-ordered pipeline: Q/K for head-pair 0 projected first, so the
  ScalarE exp backbone starts ~25us in instead of ~130us; V projection and
  later pairs' projections fill TensorE idle while exp runs.
- ancillary DMAs (rope swap, l row-move, odd-head output move) issued on
  the Pool queue (25ns issue) instead of SP/ACT queues (600-900ns).
"""

from contextlib import ExitStack

import numpy as np

import concourse.bass as bass
import concourse.tile as tile
from concourse import bacc, mybir
from concourse.bass import ds, ts
from concourse.bass_utils import run_bass_kernel_spmd

B, S, D, H, DH = 4, 2048, 1024, 16, 64
HL = 8          # heads per core
INNER = H * DH  # 1024
KC = D // 128   # 8 contraction chunks
NT = S // 128   # 16 token tiles
F32 = mybir.dt.float32
BF = mybir.dt.bfloat16

EXP = mybir.ActivationFunctionType.Exp
SCALE = 1.0 / np.sqrt(DH)


def _emit_attention(nc, pools, qkt, vsb, ot, pair, parts=None, filler=None):
    """Attention for the two heads of `pair` (local heads 2p, 2p+1)."""
    scp, pjp, pavp, nrm = (pools[k] for k in ("sc", "pj", "pav", "nrm"))
    if parts is None:
        parts = [(h, qh) for h in range(2) for qh in range(2)]
    for h, qh in parts:
        if True:
            hloc = 2 * pair + h
            q_ap = qkt[pair][ds(64 * h, 64), :]
            k_ap = qkt[4 + pair][ds(64 * h, 64), :]
            q0, q1 = 1024 * qh, 1024 * (qh + 1)
            pav = [pavp.tile([DH + 1, 512], F32, tag="pav", name=f"pav{_c}")
                   for _c in range(2)]
            jmax = 8 * (qh + 1) - 1
            for j in range(jmax + 1):
                gs = max(q0, 128 * j)       # first valid q col
                cw = q1 - gs
                ps = scp.tile([128, cw], F32, tag="sc")
                for po in range(0, cw, 512):
                    pw = min(512, cw - po)
                    nc.tensor.matmul(
                        ps[:, ds(po, pw)],
                        k_ap[:, ds(128 * j, 128)],
                        q_ap[:, ds(gs + po, pw)],
                        start=True, stop=True)
                pj = pjp.tile([128, cw], BF, tag="P")
                nc.scalar.activation(pj[:], ps[:], EXP, scale=SCALE)
                if gs == 128 * j:
                    # diagonal block: causal-mask first 128 cols
                    nc.gpsimd.affine_select(
                        out=pj[:, 0:128], in_=pj[:, 0:128],
                        compare_op=mybir.AluOpType.is_ge, fill=0.0,
                        base=0, pattern=[[1, 128]],
                        channel_multiplier=-1)
                # AV accumulate: one psum accumulator per 512 q-cols
                for c in range(2):
                    cr = q0 + 512 * c       # abs start col of region
                    cs = max(cr, 128 * j)   # abs start col this j covers
                    w = cr + 512 - cs
                    if w <= 0:
                        continue
                    nc.tensor.matmul(
                        pav[c][:, ds(cs - cr, w)],
                        vsb[:, j, hloc, 0:DH + 1],
                        pj[:, ds(cs - gs, w)],
                        start=(j == 0),
                        stop=(j == min(jmax, (cr + 511) // 128)))
                if filler is not None:
                    next(filler, None)
            # ---- normalize per region: pav[0:64] / pav[64] -> ot ----
            for c in range(2):
                cr = q0 + 512 * c
                qsl = ds(cr, 512)
                sc = nrm.tile([64, 512], BF, tag="sc")
                nc.vector.tensor_copy(sc[:], pav[c][ds(0, DH), :])
                lf = nrm.tile([128, 512], F32, tag="lf")
                nc.vector.tensor_copy(lf[ds(64, 1), :], pav[c][ds(DH, 1), :])
                nc.sync.dma_start(lf[ds(0, 1), :], lf[ds(64, 1), :])
                nc.vector.reciprocal(lf[ds(0, 1), :], lf[ds(0, 1), :])
                rb = nrm.tile([64, 512], F32, tag="rb")
                nc.gpsimd.partition_broadcast(
                    rb[:], lf[ds(0, 1), :], channels=64)
                if h == 0:
                    nc.vector.tensor_mul(ot[pair][ds(0, 64), qsl], sc[:], rb[:])
                else:
                    ott = nrm.tile([64, 512], BF, tag="ott")
                    nc.vector.tensor_mul(ott[:], sc[:], rb[:])
                    nc.sync.dma_start(ot[pair][ds(64, 64), qsl], ott[:])


def build_kernel(nc, phases=3):
    xT = nc.dram_tensor("xT", [D, S], BF, kind="ExternalInput").ap()
    wq = nc.dram_tensor("wq", [D, HL * DH], BF, kind="ExternalInput").ap()
    wk = nc.dram_tensor("wk", [D, HL * DH], BF, kind="ExternalInput").ap()
    wv = nc.dram_tensor("wv", [D, HL * DH], BF, kind="ExternalInput").ap()
    wo = nc.dram_tensor("wo", [HL * DH, D], BF, kind="ExternalInput").ap()
    cc = nc.dram_tensor("cc", [128, S], BF, kind="ExternalInput").ap()
    ssw = nc.dram_tensor("ssw", [128, S], BF, kind="ExternalInput").ap()
    y = nc.dram_tensor("y", [S, D], F32, kind="ExternalOutput").ap()

    with tile.TileContext(nc) as tc, ExitStack() as top:
        cpool = top.enter_context(tc.tile_pool(name="consts", bufs=1))
        qkpool = top.enter_context(tc.tile_pool(name="qkp", bufs=1))
        otpool = top.enter_context(tc.tile_pool(name="otp", bufs=1))

        # ---- input loads (SP queue), most-urgent first ----
        wq_sb = cpool.tile([128, KC, 512], BF, tag="wq", name="wq")
        nc.sync.dma_start(wq_sb[:], wq.rearrange("(k p) n -> p k n", p=128))
        cc_sb = cpool.tile([128, S], BF, tag="cc", name="cc")
        nc.sync.dma_start(cc_sb[:], cc[:, :])
        ssw_sb = cpool.tile([128, S], BF, tag="ssw", name="ssw")
        nc.sync.dma_start(ssw_sb[:], ssw[:, :])
        xsb = []
        for k in range(KC):
            xh = cpool.tile([128, S], BF, tag=f"x{k}", name=f"x{k}")
            nc.sync.dma_start(xh[:], xT[ts(k, 128), :])
            xsb.append(xh)
        wk_sb = cpool.tile([128, KC, 512], BF, tag="wk", name="wk")
        nc.sync.dma_start(wk_sb[:], wk.rearrange("(k p) n -> p k n", p=128))
        wv_sb = cpool.tile([128, KC, 512], BF, tag="wv", name="wv")
        nc.sync.dma_start(wv_sb[:], wv.rearrange("(k p) n -> p k n", p=128))
        wo_sb = cpool.tile([128, 4, D], BF, tag="wo", name="wo")
        nc.sync.dma_start(wo_sb[:], wo.rearrange("(k p) n -> p k n", p=128))

        qkt = [qkpool.tile([128, S], BF, tag=f"qkt{t}", name=f"qkt{t}")
               for t in range(8)]
        vsb = qkpool.tile([128, NT, HL, DH + 1], BF, tag="vsb", name="vsb")
        ot = [otpool.tile([128, S], BF, tag=f"ot{t}", name=f"ot{t}")
              for t in range(4)]

        nc.gpsimd.memset(vsb[:, :, :, DH], 1.0)
        # pre-warm the exp table set while projections run
        warm = cpool.tile([1, 16], F32, tag="warm", name="warm")
        nc.gpsimd.memset(warm[:], 0.0)
        nc.scalar.activation(warm[:], warm[:], EXP, scale=1.0)

        with ExitStack() as mid:
            rtmp = mid.enter_context(tc.tile_pool(name="rtmp", bufs=4))
            scp = mid.enter_context(
                tc.tile_pool(name="scp", bufs=2, space="PSUM"))
            pjp = mid.enter_context(tc.tile_pool(name="pjp", bufs=6))
            pavp = mid.enter_context(
                tc.tile_pool(name="pavp", bufs=2, space="PSUM"))
            nrm = mid.enter_context(tc.tile_pool(name="nrm", bufs=2))
            projscope = mid.enter_context(ExitStack())
            psp = projscope.enter_context(
                tc.tile_pool(name="psp", bufs=2, space="PSUM"))
            pools = dict(sc=scp, pj=pjp, pav=pavp, nrm=nrm)

            def project_qk(pair, chs=range(4)):
                """Q,K projection + rope for one head-pair, 512-col chunks."""
                for wsb, toff in ((wq_sb, pair), (wk_sb, 4 + pair)):
                    for ch in chs:            # 512-token chunks
                        off = 512 * ch
                        ps = psp.tile([128, 512], F32, tag="ps512")
                        for k in range(KC):
                            nc.tensor.matmul(
                                ps[:], wsb[:, k, ts(pair, 128)],
                                xsb[k][:, ds(off, 512)],
                                start=(k == 0), stop=(k == KC - 1))
                        sl = ds(off, 512)
                        nc.vector.tensor_mul(
                            qkt[toff][:, sl], ps[:], cc_sb[:, sl])
                        v2 = rtmp.tile([128, 512], BF, tag="v2")
                        nc.vector.tensor_mul(v2[:], ps[:], ssw_sb[:, sl])
                        v2s = rtmp.tile([128, 512], BF, tag="v2s")
                        for blk in range(4):
                            src = (blk ^ 1) * 32
                            nc.sync.dma_start(
                                v2s[ds(blk * 32, 32), :], v2[ds(src, 32), :])
                        nc.vector.tensor_tensor(
                            qkt[toff][:, sl], qkt[toff][:, sl], v2s[:],
                            op=mybir.AluOpType.add)

            def project_v(tts):
                for tt in tts:
                    psV = psp.tile([128, 512], F32, tag="ps512")
                    for k in range(KC):
                        nc.tensor.matmul(
                            psV[:], xsb[k][:, ts(tt, 128)], wv_sb[:, k, :],
                            start=(k == 0), stop=(k == KC - 1))
                    nc.scalar.copy(
                        vsb[:, tt, :, 0:DH],
                        psV[:].rearrange("p (h d) -> p h d", h=HL))

            project_qk(0, chs=[0, 1])
            if phases >= 2:
                project_v(range(0, 8))
                _emit_attention(nc, pools, qkt, vsb, ot, 0, parts=[(0, 0)])
                project_qk(0, chs=[2, 3])
                project_v(range(8, 16))
                _emit_attention(nc, pools, qkt, vsb, ot, 0,
                                parts=[(0, 1), (1, 0), (1, 1)])
                for pair in range(1, 4):
                    project_qk(pair)
                    _emit_attention(nc, pools, qkt, vsb, ot, pair)
            else:
                project_v(range(0, 8))
                project_qk(0, chs=[2, 3])
                project_v(range(8, 16))
                for pair in range(1, 4):
                    project_qk(pair)

        # ---------------- out projection -----------------------------
        if phases < 3:
            return nc
        with ExitStack() as ph:
            ypool = ph.enter_context(tc.tile_pool(name="ypool", bufs=3))
            psy = ph.enter_context(
                tc.tile_pool(name="psy", bufs=3, space="PSUM"))
            for tt in range(NT):
                ps = psy.tile([128, D], F32, tag="psy")
                for k in range(4):
                    for half in range(2):
                        nc.tensor.matmul(
                            ps[:, ts(half, 512)],
                            ot[k][:, ts(tt, 128)],
                            wo_sb[:, k, ts(half, 512)],
                            start=(k == 0), stop=(k == 3))
                ysb = ypool.tile([128, D], F32, tag="y")
                nc.scalar.copy(ysb[:], ps[:])
                nc.sync.dma_start(y[ts(tt, 128), :], ysb[:])
    return nc


# ---------------- host side ------------------------------------------------

def _rope_tables():
    i = np.arange(DH // 2, dtype=np.float32)
    thetas = np.power(np.float32(10000.0), -2.0 * (i - 1.0) / DH)
    vals = thetas[:, None].astype(np.float32) * \
        np.arange(S, dtype=np.float32)[None, :]
    cos32 = np.cos(vals).astype(np.float32)
    sin32 = np.sin(vals).astype(np.float32)
    CC = np.tile(cos32, (4, 1))
    SSsw = np.concatenate([sin32, -sin32, sin32, -sin32], axis=0)
    return np.ascontiguousarray(CC), np.ascontiguousarray(SSsw)


def _qk_col_perm(g):
    cols = []
    for m in range(4):
        for hh in (2 * m, 2 * m + 1):
            hg = HL * g + hh
            cols += [hg * DH + 2 * i for i in range(32)]
            cols += [hg * DH + 2 * i + 1 for i in range(32)]
    return np.array(cols)


_CACHE = {}


def _get_module():
    if "nc" not in _CACHE:
        nc = bacc.Bacc("TRN2", target_bir_lowering=False, debug=False,
                       num_devices=8)
        build_kernel(nc)
        nc.compile()
        _CACHE["nc"] = nc
    return _CACHE["nc"]


def make_in_maps(x, Wqkv, Wout):
    import ml_dtypes
    bf16 = ml_dtypes.bfloat16
    x = np.asarray(x, np.float32)
    Wqkv = np.asarray(Wqkv, np.float32)
    Wout = np.asarray(Wout, np.float32)
    CC, SSsw = _rope_tables()
    shard = {}
    for g in range(2):
        perm = _qk_col_perm(g)
        vcols = np.arange(HL * g * DH, HL * (g + 1) * DH)
        shard[g] = dict(
            wq=np.ascontiguousarray(
                Wqkv[:, 0 * INNER:1 * INNER][:, perm].astype(bf16)),
            wk=np.ascontiguousarray(
                Wqkv[:, 1 * INNER:2 * INNER][:, perm].astype(bf16)),
            wv=np.ascontiguousarray(
                Wqkv[:, 2 * INNER:3 * INNER][:, vcols].astype(bf16)),
            wo=np.ascontiguousarray(Wout[vcols, :].astype(bf16)),
        )
    in_maps = []
    for c in range(8):
        b, g = c // 2, c % 2
        in_maps.append(dict(
            xT=np.ascontiguousarray(x[b].T.astype(bf16)),
            cc=CC.astype(bf16), ssw=SSsw.astype(bf16), **shard[g]))
    return in_maps


def kernel(x, Wqkv, Wout, bout):
    bout = np.asarray(bout, np.float32)
    nc = _get_module()
    in_maps = make_in_maps(x, Wqkv, Wout)
    res = run_bass_kernel_spmd(nc, in_maps, core_ids=list(range(8)))
    ys = [np.asarray(r["y"], dtype=np.float32) for r in res.results]
    out = np.stack([ys[2 * b] + ys[2 * b + 1] + bout for b in range(B)])
    return out.astype(np.float32)


# revision 5
# speedup vs baseline: 1.4548x; 1.0034x over previous
"""Trainium2 Bass kernel v2 for nn_Attention (qkv proj + RoPE + causal
attention + out proj), tensor-parallel over 8 NeuronCores: core c handles
batch b=c//2, head-group g=c%2 (8 heads).

v2 changes vs baseline:
- bf16 activations/weights end-to-end (fp32 psum accumulation, fp32 rope
  tables, fp32 softmax denominators) -> half the DMA bytes, 2x DVE adds,
  no fp32r small-N matmul penalty.
- pair-ordered pipeline: Q/K for head-pair 0 projected first, so the
  ScalarE exp backbone starts ~25us in instead of ~130us; V projection and
  later pairs' projections fill TensorE idle while exp runs.
- ancillary DMAs (rope swap, l row-move, odd-head output move) issued on
  the Pool queue (25ns issue) instead of SP/ACT queues (600-900ns).
"""

from contextlib import ExitStack

import numpy as np

import concourse.bass as bass
import concourse.tile as tile
from concourse import bacc, mybir
from concourse.bass import ds, ts
from concourse.bass_utils import run_bass_kernel_spmd

B, S, D, H, DH = 4, 2048, 1024, 16, 64
HL = 8          # heads per core
INNER = H * DH  # 1024
KC = D // 128   # 8 contraction chunks
NT = S // 128   # 16 token tiles
F32 = mybir.dt.float32
BF = mybir.dt.bfloat16

EXP = mybir.ActivationFunctionType.Exp
SCALE = 1.0 / np.sqrt(DH)


def _emit_attention(nc, pools, qkt, vsb, ot, pair, parts=None, filler=None):
    """Attention for the two heads of `pair` (local heads 2p, 2p+1)."""
    scp, pjp, pavp, nrm = (pools[k] for k in ("sc", "pj", "pav", "nrm"))
    if parts is None:
        parts = [(h, qh) for h in range(2) for qh in range(2)]
    for h, qh in parts:
        if True:
            hloc = 2 * pair + h
            q_ap = qkt[pair][ds(64 * h, 64), :]
            k_ap = qkt[4 + pair][ds(64 * h, 64), :]
            q0, q1 = 1024 * qh, 1024 * (qh + 1)
            pav = [pavp.tile([DH + 1, 512], F32, tag="pav", name=f"pav{_c}")
                   for _c in range(2)]
            jmax = 8 * (qh + 1) - 1
            for j in range(jmax + 1):
                gs = max(q0, 128 * j)       # first valid q col
                cw = q1 - gs
                ps = scp.tile([128, cw], F32, tag="sc")
                for po in range(0, cw, 512):
                    pw = min(512, cw - po)
                    nc.tensor.matmul(
                        ps[:, ds(po, pw)],
                        k_ap[:, ds(128 * j, 128)],
                        q_ap[:, ds(gs + po, pw)],
                        start=True, stop=True)
                pj = pjp.tile([128, cw], BF, tag="P")
                nc.scalar.activation(pj[:], ps[:], EXP, scale=SCALE)
                if gs == 128 * j:
                    # diagonal block: causal-mask first 128 cols
                    nc.gpsimd.affine_select(
                        out=pj[:, 0:128], in_=pj[:, 0:128],
                        compare_op=mybir.AluOpType.is_ge, fill=0.0,
                        base=0, pattern=[[1, 128]],
                        channel_multiplier=-1)
                # AV accumulate: one psum accumulator per 512 q-cols
                for c in range(2):
                    cr = q0 + 512 * c       # abs start col of region
                    cs = max(cr, 128 * j)   # abs start col this j covers
                    w = cr + 512 - cs
                    if w <= 0:
                        continue
                    nc.tensor.matmul(
                        pav[c][:, ds(cs - cr, w)],
                        vsb[:, j, hloc, 0:DH + 1],
                        pj[:, ds(cs - gs, w)],
                        start=(j == 0),
                        stop=(j == min(jmax, (cr + 511) // 128)))
                if filler is not None:
                    next(filler, None)
            # ---- normalize per region: pav[0:64] / pav[64] -> ot ----
            for c in range(2):
                cr = q0 + 512 * c
                qsl = ds(cr, 512)
                sc = nrm.tile([64, 512], BF, tag="sc")
                nc.vector.tensor_copy(sc[:], pav[c][ds(0, DH), :])
                lf = nrm.tile([128, 512], F32, tag="lf")
                nc.vector.tensor_copy(lf[ds(64, 1), :], pav[c][ds(DH, 1), :])
                nc.sync.dma_start(lf[ds(0, 1), :], lf[ds(64, 1), :])
                nc.vector.reciprocal(lf[ds(0, 1), :], lf[ds(0, 1), :])
                rb = nrm.tile([64, 512], F32, tag="rb")
                nc.gpsimd.partition_broadcast(
                    rb[:], lf[ds(0, 1), :], channels=64)
                if h == 0:
                    nc.vector.tensor_mul(ot[pair][ds(0, 64), qsl], sc[:], rb[:])
                else:
                    ott = nrm.tile([64, 512], BF, tag="ott")
                    nc.vector.tensor_mul(ott[:], sc[:], rb[:])
                    nc.sync.dma_start(ot[pair][ds(64, 64), qsl], ott[:])


def build_kernel(nc, phases=3):
    xT = nc.dram_tensor("xT", [D, S], BF, kind="ExternalInput").ap()
    wq = nc.dram_tensor("wq", [D, HL * DH], BF, kind="ExternalInput").ap()
    wk = nc.dram_tensor("wk", [D, HL * DH], BF, kind="ExternalInput").ap()
    wv = nc.dram_tensor("wv", [D, HL * DH], BF, kind="ExternalInput").ap()
    wo = nc.dram_tensor("wo", [HL * DH, D], BF, kind="ExternalInput").ap()
    cc = nc.dram_tensor("cc", [128, S], BF, kind="ExternalInput").ap()
    ssw = nc.dram_tensor("ssw", [128, S], BF, kind="ExternalInput").ap()
    y = nc.dram_tensor("y", [S, D], F32, kind="ExternalOutput").ap()

    with tile.TileContext(nc) as tc, ExitStack() as top:
        cpool = top.enter_context(tc.tile_pool(name="consts", bufs=1))
        qkpool = top.enter_context(tc.tile_pool(name="qkp", bufs=1))
        otpool = top.enter_context(tc.tile_pool(name="otp", bufs=1))

        # ---- input loads (SP queue), most-urgent first ----
        wqr = wq.rearrange("(k p) n -> p k n", p=128)
        wkr = wk.rearrange("(k p) n -> p k n", p=128)
        wq_sb = cpool.tile([128, KC, 512], BF, tag="wq", name="wq")
        nc.sync.dma_start(wq_sb[:, :, 0:128], wqr[:, :, 0:128])
        cc_sb = cpool.tile([128, S], BF, tag="cc", name="cc")
        nc.sync.dma_start(cc_sb[:], cc[:, :])
        ssw_sb = cpool.tile([128, S], BF, tag="ssw", name="ssw")
        nc.sync.dma_start(ssw_sb[:], ssw[:, :])
        xsb = []
        for k in range(KC):
            xh = cpool.tile([128, S], BF, tag=f"x{k}", name=f"x{k}")
            nc.sync.dma_start(xh[:, ds(0, 1024)], xT[ts(k, 128), ds(0, 1024)])
            xsb.append(xh)
        wk_sb = cpool.tile([128, KC, 512], BF, tag="wk", name="wk")
        nc.sync.dma_start(wk_sb[:, :, 0:128], wkr[:, :, 0:128])
        wv_sb = cpool.tile([128, KC, 512], BF, tag="wv", name="wv")
        nc.sync.dma_start(wv_sb[:], wv.rearrange("(k p) n -> p k n", p=128))
        nc.sync.dma_start(wq_sb[:, :, 128:512], wqr[:, :, 128:512])
        nc.sync.dma_start(wk_sb[:, :, 128:512], wkr[:, :, 128:512])
        for k in range(KC):
            nc.sync.dma_start(
                xsb[k][:, ds(1024, 1024)], xT[ts(k, 128), ds(1024, 1024)])
        wo_sb = cpool.tile([128, 4, D], BF, tag="wo", name="wo")
        nc.sync.dma_start(wo_sb[:], wo.rearrange("(k p) n -> p k n", p=128))

        qkt = [qkpool.tile([128, S], BF, tag=f"qkt{t}", name=f"qkt{t}")
               for t in range(8)]
        vsb = qkpool.tile([128, NT, HL, DH + 1], BF, tag="vsb", name="vsb")
        ot = [otpool.tile([128, S], BF, tag=f"ot{t}", name=f"ot{t}")
              for t in range(4)]

        nc.gpsimd.memset(vsb[:, :, :, DH], 1.0)
        # pre-warm the exp table set while projections run
        warm = cpool.tile([1, 16], F32, tag="warm", name="warm")
        nc.gpsimd.memset(warm[:], 0.0)
        nc.scalar.activation(warm[:], warm[:], EXP, scale=1.0)

        with ExitStack() as mid:
            rtmp = mid.enter_context(tc.tile_pool(name="rtmp", bufs=4))
            scp = mid.enter_context(
                tc.tile_pool(name="scp", bufs=2, space="PSUM"))
            pjp = mid.enter_context(tc.tile_pool(name="pjp", bufs=6))
            pavp = mid.enter_context(
                tc.tile_pool(name="pavp", bufs=2, space="PSUM"))
            nrm = mid.enter_context(tc.tile_pool(name="nrm", bufs=2))
            projscope = mid.enter_context(ExitStack())
            psp = projscope.enter_context(
                tc.tile_pool(name="psp", bufs=2, space="PSUM"))
            pools = dict(sc=scp, pj=pjp, pav=pavp, nrm=nrm)

            def project_qk(pair, chs=range(4)):
                """Q,K projection + rope for one head-pair, 512-col chunks."""
                for wsb, toff in ((wq_sb, pair), (wk_sb, 4 + pair)):
                    for ch in chs:            # 512-token chunks
                        off = 512 * ch
                        ps = psp.tile([128, 512], F32, tag="ps512")
                        for k in range(KC):
                            nc.tensor.matmul(
                                ps[:], wsb[:, k, ts(pair, 128)],
                                xsb[k][:, ds(off, 512)],
                                start=(k == 0), stop=(k == KC - 1))
                        sl = ds(off, 512)
                        nc.vector.tensor_mul(
                            qkt[toff][:, sl], ps[:], cc_sb[:, sl])
                        v2 = rtmp.tile([128, 512], BF, tag="v2")
                        nc.vector.tensor_mul(v2[:], ps[:], ssw_sb[:, sl])
                        v2s = rtmp.tile([128, 512], BF, tag="v2s")
                        for blk in range(4):
                            src = (blk ^ 1) * 32
                            nc.sync.dma_start(
                                v2s[ds(blk * 32, 32), :], v2[ds(src, 32), :])
                        nc.vector.tensor_tensor(
                            qkt[toff][:, sl], qkt[toff][:, sl], v2s[:],
                            op=mybir.AluOpType.add)

            def project_v(tts):
                for tt in tts:
                    psV = psp.tile([128, 512], F32, tag="ps512")
                    for k in range(KC):
                        nc.tensor.matmul(
                            psV[:], xsb[k][:, ts(tt, 128)], wv_sb[:, k, :],
                            start=(k == 0), stop=(k == KC - 1))
                    nc.scalar.copy(
                        vsb[:, tt, :, 0:DH],
                        psV[:].rearrange("p (h d) -> p h d", h=HL))

            project_qk(0, chs=[0, 1])
            if phases >= 2:
                project_v(range(0, 8))
                _emit_attention(nc, pools, qkt, vsb, ot, 0, parts=[(0, 0)])
                project_qk(0, chs=[2, 3])
                project_v(range(8, 16))
                _emit_attention(nc, pools, qkt, vsb, ot, 0,
                                parts=[(0, 1), (1, 0), (1, 1)])
                for pair in range(1, 4):
                    project_qk(pair)
                    _emit_attention(nc, pools, qkt, vsb, ot, pair)
            else:
                project_v(range(0, 8))
                project_qk(0, chs=[2, 3])
                project_v(range(8, 16))
                for pair in range(1, 4):
                    project_qk(pair)

        # ---------------- out projection -----------------------------
        if phases < 3:
            return nc
        with ExitStack() as ph:
            ypool = ph.enter_context(tc.tile_pool(name="ypool", bufs=3))
            psy = ph.enter_context(
                tc.tile_pool(name="psy", bufs=3, space="PSUM"))
            for tt in range(NT):
                ps = psy.tile([128, D], F32, tag="psy")
                for k in range(4):
                    for half in range(2):
                        nc.tensor.matmul(
                            ps[:, ts(half, 512)],
                            ot[k][:, ts(tt, 128)],
                            wo_sb[:, k, ts(half, 512)],
                            start=(k == 0), stop=(k == 3))
                ysb = ypool.tile([128, D], F32, tag="y")
                nc.scalar.copy(ysb[:], ps[:])
                nc.sync.dma_start(y[ts(tt, 128), :], ysb[:])
    return nc


# ---------------- host side ------------------------------------------------

def _rope_tables():
    i = np.arange(DH // 2, dtype=np.float32)
    thetas = np.power(np.float32(10000.0), -2.0 * (i - 1.0) / DH)
    vals = thetas[:, None].astype(np.float32) * \
        np.arange(S, dtype=np.float32)[None, :]
    cos32 = np.cos(vals).astype(np.float32)
    sin32 = np.sin(vals).astype(np.float32)
    CC = np.tile(cos32, (4, 1))
    SSsw = np.concatenate([sin32, -sin32, sin32, -sin32], axis=0)
    return np.ascontiguousarray(CC), np.ascontiguousarray(SSsw)


def _qk_col_perm(g):
    cols = []
    for m in range(4):
        for hh in (2 * m, 2 * m + 1):
            hg = HL * g + hh
            cols += [hg * DH + 2 * i for i in range(32)]
            cols += [hg * DH + 2 * i + 1 for i in range(32)]
    return np.array(cols)


_CACHE = {}


def _get_module():
    if "nc" not in _CACHE:
        nc = bacc.Bacc("TRN2", target_bir_lowering=False, debug=False,
                       num_devices=8)
        build_kernel(nc)
        nc.compile()
        _CACHE["nc"] = nc
    return _CACHE["nc"]


def make_in_maps(x, Wqkv, Wout):
    import ml_dtypes
    bf16 = ml_dtypes.bfloat16
    x = np.asarray(x, np.float32)
    Wqkv = np.asarray(Wqkv, np.float32)
    Wout = np.asarray(Wout, np.float32)
    CC, SSsw = _rope_tables()
    shard = {}
    for g in range(2):
        perm = _qk_col_perm(g)
        vcols = np.arange(HL * g * DH, HL * (g + 1) * DH)
        shard[g] = dict(
            wq=np.ascontiguousarray(
                Wqkv[:, 0 * INNER:1 * INNER][:, perm].astype(bf16)),
            wk=np.ascontiguousarray(
                Wqkv[:, 1 * INNER:2 * INNER][:, perm].astype(bf16)),
            wv=np.ascontiguousarray(
                Wqkv[:, 2 * INNER:3 * INNER][:, vcols].astype(bf16)),
            wo=np.ascontiguousarray(Wout[vcols, :].astype(bf16)),
        )
    in_maps = []
    for c in range(8):
        b, g = c // 2, c % 2
        in_maps.append(dict(
            xT=np.ascontiguousarray(x[b].T.astype(bf16)),
            cc=CC.astype(bf16), ssw=SSsw.astype(bf16), **shard[g]))
    return in_maps


def kernel(x, Wqkv, Wout, bout):
    bout = np.asarray(bout, np.float32)
    nc = _get_module()
    in_maps = make_in_maps(x, Wqkv, Wout)
    res = run_bass_kernel_spmd(nc, in_maps, core_ids=list(range(8)))
    ys = [np.asarray(r["y"], dtype=np.float32) for r in res.results]
    out = np.stack([ys[2 * b] + ys[2 * b + 1] + bout for b in range(B)])
    return out.astype(np.float32)


# revision 6
# speedup vs baseline: 1.4781x; 1.0160x over previous
"""Trainium2 Bass kernel v2 for nn_Attention (qkv proj + RoPE + causal
attention + out proj), tensor-parallel over 8 NeuronCores: core c handles
batch b=c//2, head-group g=c%2 (8 heads).

v2 changes vs baseline:
- bf16 activations/weights end-to-end (fp32 psum accumulation, fp32 rope
  tables, fp32 softmax denominators) -> half the DMA bytes, 2x DVE adds,
  no fp32r small-N matmul penalty.
- pair-ordered pipeline: Q/K for head-pair 0 projected first, so the
  ScalarE exp backbone starts ~25us in instead of ~130us; V projection and
  later pairs' projections fill TensorE idle while exp runs.
- ancillary DMAs (rope swap, l row-move, odd-head output move) issued on
  the Pool queue (25ns issue) instead of SP/ACT queues (600-900ns).
"""

from contextlib import ExitStack

import numpy as np

import concourse.bass as bass
import concourse.tile as tile
from concourse import bacc, mybir
from concourse.bass import ds, ts
from concourse.bass_utils import run_bass_kernel_spmd

B, S, D, H, DH = 4, 2048, 1024, 16, 64
HL = 8          # heads per core
INNER = H * DH  # 1024
KC = D // 128   # 8 contraction chunks
NT = S // 128   # 16 token tiles
F32 = mybir.dt.float32
BF = mybir.dt.bfloat16

EXP = mybir.ActivationFunctionType.Exp
SCALE = 1.0 / np.sqrt(DH)


def _emit_attention(nc, pools, qkt, vsb, ot, pair, parts=None, filler=None):
    """Attention for the two heads of `pair` (local heads 2p, 2p+1)."""
    scp, pjp, pavp, nrm = (pools[k] for k in ("sc", "pj", "pav", "nrm"))
    if parts is None:
        parts = [(h, qh) for h in range(2) for qh in range(2)]
    for h, qh in parts:
        if True:
            hloc = 2 * pair + h
            q_ap = qkt[pair][ds(64 * h, 64), :]
            k_ap = qkt[4 + pair][ds(64 * h, 64), :]
            q0, q1 = 1024 * qh, 1024 * (qh + 1)
            pav = [pavp.tile([DH + 1, 512], F32, tag="pav", name=f"pav{_c}")
                   for _c in range(2)]
            jmax = 8 * (qh + 1) - 1
            for j in range(jmax + 1):
                gs = max(q0, 128 * j)       # first valid q col
                cw = q1 - gs
                ps = scp.tile([128, cw], F32, tag="sc")
                for po in range(0, cw, 512):
                    pw = min(512, cw - po)
                    nc.tensor.matmul(
                        ps[:, ds(po, pw)],
                        k_ap[:, ds(128 * j, 128)],
                        q_ap[:, ds(gs + po, pw)],
                        start=True, stop=True)
                pj = pjp.tile([128, cw], BF, tag="P")
                nc.scalar.activation(pj[:], ps[:], EXP, scale=SCALE)
                if gs == 128 * j:
                    # diagonal block: causal-mask first 128 cols
                    nc.gpsimd.affine_select(
                        out=pj[:, 0:128], in_=pj[:, 0:128],
                        compare_op=mybir.AluOpType.is_ge, fill=0.0,
                        base=0, pattern=[[1, 128]],
                        channel_multiplier=-1)
                # AV accumulate: one psum accumulator per 512 q-cols
                for c in range(2):
                    cr = q0 + 512 * c       # abs start col of region
                    cs = max(cr, 128 * j)   # abs start col this j covers
                    w = cr + 512 - cs
                    if w <= 0:
                        continue
                    nc.tensor.matmul(
                        pav[c][:, ds(cs - cr, w)],
                        vsb[:, j, hloc, 0:DH + 1],
                        pj[:, ds(cs - gs, w)],
                        start=(j == 0),
                        stop=(j == min(jmax, (cr + 511) // 128)))
                if filler is not None:
                    next(filler, None)
            # ---- normalize per region: pav[0:64] / pav[64] -> ot ----
            for c in range(2):
                cr = q0 + 512 * c
                qsl = ds(cr, 512)
                sc = nrm.tile([64, 512], BF, tag="sc")
                nc.vector.tensor_copy(sc[:], pav[c][ds(0, DH), :])
                lf = nrm.tile([128, 512], F32, tag="lf")
                nc.vector.tensor_copy(lf[ds(64, 1), :], pav[c][ds(DH, 1), :])
                nc.sync.dma_start(lf[ds(0, 1), :], lf[ds(64, 1), :])
                nc.vector.reciprocal(lf[ds(0, 1), :], lf[ds(0, 1), :])
                lb = nrm.tile([1, 512], BF, tag="lb")
                nc.vector.tensor_copy(lb[:], lf[ds(0, 1), :])
                rb = nrm.tile([64, 512], BF, tag="rb")
                nc.gpsimd.partition_broadcast(
                    rb[:], lb[:], channels=64)
                if h == 0:
                    nc.vector.tensor_mul(ot[pair][ds(0, 64), qsl], sc[:], rb[:])
                else:
                    ott = nrm.tile([64, 512], BF, tag="ott")
                    nc.vector.tensor_mul(ott[:], sc[:], rb[:])
                    nc.sync.dma_start(ot[pair][ds(64, 64), qsl], ott[:])


def build_kernel(nc, phases=3):
    xT = nc.dram_tensor("xT", [D, S], BF, kind="ExternalInput").ap()
    wq = nc.dram_tensor("wq", [D, HL * DH], BF, kind="ExternalInput").ap()
    wk = nc.dram_tensor("wk", [D, HL * DH], BF, kind="ExternalInput").ap()
    wv = nc.dram_tensor("wv", [D, HL * DH], BF, kind="ExternalInput").ap()
    wo = nc.dram_tensor("wo", [HL * DH, D], BF, kind="ExternalInput").ap()
    cc = nc.dram_tensor("cc", [128, S], BF, kind="ExternalInput").ap()
    ssw = nc.dram_tensor("ssw", [128, S], BF, kind="ExternalInput").ap()
    y = nc.dram_tensor("y", [S, D], F32, kind="ExternalOutput").ap()

    with tile.TileContext(nc) as tc, ExitStack() as top:
        cpool = top.enter_context(tc.tile_pool(name="consts", bufs=1))
        qkpool = top.enter_context(tc.tile_pool(name="qkp", bufs=1))
        otpool = top.enter_context(tc.tile_pool(name="otp", bufs=1))

        # ---- input loads (SP queue), most-urgent first ----
        wqr = wq.rearrange("(k p) n -> p k n", p=128)
        wkr = wk.rearrange("(k p) n -> p k n", p=128)
        wq_sb = cpool.tile([128, KC, 512], BF, tag="wq", name="wq")
        nc.sync.dma_start(wq_sb[:, :, 0:128], wqr[:, :, 0:128])
        cc_sb = cpool.tile([128, S], BF, tag="cc", name="cc")
        nc.sync.dma_start(cc_sb[:], cc[:, :])
        ssw_sb = cpool.tile([128, S], BF, tag="ssw", name="ssw")
        nc.sync.dma_start(ssw_sb[:], ssw[:, :])
        xsb = []
        for k in range(KC):
            xh = cpool.tile([128, S], BF, tag=f"x{k}", name=f"x{k}")
            nc.sync.dma_start(xh[:, ds(0, 1024)], xT[ts(k, 128), ds(0, 1024)])
            xsb.append(xh)
        wk_sb = cpool.tile([128, KC, 512], BF, tag="wk", name="wk")
        nc.sync.dma_start(wk_sb[:, :, 0:128], wkr[:, :, 0:128])
        wv_sb = cpool.tile([128, KC, 512], BF, tag="wv", name="wv")
        nc.sync.dma_start(wv_sb[:], wv.rearrange("(k p) n -> p k n", p=128))
        nc.sync.dma_start(wq_sb[:, :, 128:512], wqr[:, :, 128:512])
        nc.sync.dma_start(wk_sb[:, :, 128:512], wkr[:, :, 128:512])
        for k in range(KC):
            nc.sync.dma_start(
                xsb[k][:, ds(1024, 1024)], xT[ts(k, 128), ds(1024, 1024)])
        wo_sb = cpool.tile([128, 4, D], BF, tag="wo", name="wo")
        nc.sync.dma_start(wo_sb[:], wo.rearrange("(k p) n -> p k n", p=128))

        qkt = [qkpool.tile([128, S], BF, tag=f"qkt{t}", name=f"qkt{t}")
               for t in range(8)]
        vsb = qkpool.tile([128, NT, HL, DH + 1], BF, tag="vsb", name="vsb")
        ot = [otpool.tile([128, S], BF, tag=f"ot{t}", name=f"ot{t}")
              for t in range(4)]

        nc.gpsimd.memset(vsb[:, :, :, DH], 1.0)
        # pre-warm the exp table set while projections run
        warm = cpool.tile([1, 16], F32, tag="warm", name="warm")
        nc.gpsimd.memset(warm[:], 0.0)
        nc.scalar.activation(warm[:], warm[:], EXP, scale=1.0)

        with ExitStack() as mid:
            rtmp = mid.enter_context(tc.tile_pool(name="rtmp", bufs=4))
            scp = mid.enter_context(
                tc.tile_pool(name="scp", bufs=2, space="PSUM"))
            pjp = mid.enter_context(tc.tile_pool(name="pjp", bufs=6))
            pavp = mid.enter_context(
                tc.tile_pool(name="pavp", bufs=2, space="PSUM"))
            nrm = mid.enter_context(tc.tile_pool(name="nrm", bufs=2))
            projscope = mid.enter_context(ExitStack())
            psp = projscope.enter_context(
                tc.tile_pool(name="psp", bufs=2, space="PSUM"))
            pools = dict(sc=scp, pj=pjp, pav=pavp, nrm=nrm)

            def project_qk(pair, chs=range(4)):
                """Q,K projection + rope for one head-pair, 512-col chunks."""
                for wsb, toff in ((wq_sb, pair), (wk_sb, 4 + pair)):
                    for ch in chs:            # 512-token chunks
                        off = 512 * ch
                        ps = psp.tile([128, 512], F32, tag="ps512")
                        for k in range(KC):
                            nc.tensor.matmul(
                                ps[:], wsb[:, k, ts(pair, 128)],
                                xsb[k][:, ds(off, 512)],
                                start=(k == 0), stop=(k == KC - 1))
                        sl = ds(off, 512)
                        nc.vector.tensor_mul(
                            qkt[toff][:, sl], ps[:], cc_sb[:, sl])
                        v2 = rtmp.tile([128, 512], BF, tag="v2")
                        nc.vector.tensor_mul(v2[:], ps[:], ssw_sb[:, sl])
                        v2s = rtmp.tile([128, 512], BF, tag="v2s")
                        for blk in range(4):
                            src = (blk ^ 1) * 32
                            nc.sync.dma_start(
                                v2s[ds(blk * 32, 32), :], v2[ds(src, 32), :])
                        nc.vector.tensor_tensor(
                            qkt[toff][:, sl], qkt[toff][:, sl], v2s[:],
                            op=mybir.AluOpType.add)

            def project_v(tts):
                for tt in tts:
                    psV = psp.tile([128, 512], F32, tag="ps512")
                    for k in range(KC):
                        nc.tensor.matmul(
                            psV[:], xsb[k][:, ts(tt, 128)], wv_sb[:, k, :],
                            start=(k == 0), stop=(k == KC - 1))
                    nc.scalar.copy(
                        vsb[:, tt, :, 0:DH],
                        psV[:].rearrange("p (h d) -> p h d", h=HL))

            project_qk(0, chs=[0, 1])
            if phases >= 2:
                project_v(range(0, 8))
                _emit_attention(nc, pools, qkt, vsb, ot, 0, parts=[(0, 0)])
                project_qk(0, chs=[2, 3])
                project_v(range(8, 16))
                _emit_attention(nc, pools, qkt, vsb, ot, 0,
                                parts=[(0, 1), (1, 0), (1, 1)])
                for pair in range(1, 4):
                    project_qk(pair)
                    _emit_attention(nc, pools, qkt, vsb, ot, pair)
            else:
                project_v(range(0, 8))
                project_qk(0, chs=[2, 3])
                project_v(range(8, 16))
                for pair in range(1, 4):
                    project_qk(pair)

        # ---------------- out projection -----------------------------
        if phases < 3:
            return nc
        with ExitStack() as ph:
            ypool = ph.enter_context(tc.tile_pool(name="ypool", bufs=3))
            psy = ph.enter_context(
                tc.tile_pool(name="psy", bufs=3, space="PSUM"))
            for tt in range(NT):
                ps = psy.tile([128, D], F32, tag="psy")
                for k in range(4):
                    for half in range(2):
                        nc.tensor.matmul(
                            ps[:, ts(half, 512)],
                            ot[k][:, ts(tt, 128)],
                            wo_sb[:, k, ts(half, 512)],
                            start=(k == 0), stop=(k == 3))
                ysb = ypool.tile([128, D], F32, tag="y")
                nc.scalar.copy(ysb[:], ps[:])
                nc.sync.dma_start(y[ts(tt, 128), :], ysb[:])
    return nc


# ---------------- host side ------------------------------------------------

def _rope_tables():
    i = np.arange(DH // 2, dtype=np.float32)
    thetas = np.power(np.float32(10000.0), -2.0 * (i - 1.0) / DH)
    vals = thetas[:, None].astype(np.float32) * \
        np.arange(S, dtype=np.float32)[None, :]
    cos32 = np.cos(vals).astype(np.float32)
    sin32 = np.sin(vals).astype(np.float32)
    CC = np.tile(cos32, (4, 1))
    SSsw = np.concatenate([sin32, -sin32, sin32, -sin32], axis=0)
    return np.ascontiguousarray(CC), np.ascontiguousarray(SSsw)


def _qk_col_perm(g):
    cols = []
    for m in range(4):
        for hh in (2 * m, 2 * m + 1):
            hg = HL * g + hh
            cols += [hg * DH + 2 * i for i in range(32)]
            cols += [hg * DH + 2 * i + 1 for i in range(32)]
    return np.array(cols)


_CACHE = {}


def _get_module():
    if "nc" not in _CACHE:
        nc = bacc.Bacc("TRN2", target_bir_lowering=False, debug=False,
                       num_devices=8)
        build_kernel(nc)
        nc.compile()
        _CACHE["nc"] = nc
    return _CACHE["nc"]


def make_in_maps(x, Wqkv, Wout):
    import ml_dtypes
    bf16 = ml_dtypes.bfloat16
    x = np.asarray(x, np.float32)
    Wqkv = np.asarray(Wqkv, np.float32)
    Wout = np.asarray(Wout, np.float32)
    CC, SSsw = _rope_tables()
    shard = {}
    for g in range(2):
        perm = _qk_col_perm(g)
        vcols = np.arange(HL * g * DH, HL * (g + 1) * DH)
        shard[g] = dict(
            wq=np.ascontiguousarray(
                Wqkv[:, 0 * INNER:1 * INNER][:, perm].astype(bf16)),
            wk=np.ascontiguousarray(
                Wqkv[:, 1 * INNER:2 * INNER][:, perm].astype(bf16)),
            wv=np.ascontiguousarray(
                Wqkv[:, 2 * INNER:3 * INNER][:, vcols].astype(bf16)),
            wo=np.ascontiguousarray(Wout[vcols, :].astype(bf16)),
        )
    in_maps = []
    for c in range(8):
        b, g = c // 2, c % 2
        in_maps.append(dict(
            xT=np.ascontiguousarray(x[b].T.astype(bf16)),
            cc=CC.astype(bf16), ssw=SSsw.astype(bf16), **shard[g]))
    return in_maps


def kernel(x, Wqkv, Wout, bout):
    bout = np.asarray(bout, np.float32)
    nc = _get_module()
    in_maps = make_in_maps(x, Wqkv, Wout)
    res = run_bass_kernel_spmd(nc, in_maps, core_ids=list(range(8)))
    ys = [np.asarray(r["y"], dtype=np.float32) for r in res.results]
    out = np.stack([ys[2 * b] + ys[2 * b + 1] + bout for b in range(B)])
    return out.astype(np.float32)
